# revision 16
# baseline (speedup 1.0000x reference)
"""GAT attention head (single head) distributed across 8 TRN2 NeuronCores.

Math (reference):
    sf   = seq @ W1                        # [N, O]
    f1   = sf @ a1 + b1                    # [N, 1]
    f2   = sf @ a2 + b2                    # [N, 1]
    lg   = f1 + f2.T                       # [N, N]
    co   = softmax(leaky_relu(lg, 0.2) + bias, axis=-1)
    out  = elu(co @ sf)                    # [N, O]

Key algebraic transform: with S = f1[r] + f2[n],
    leaky_relu(S, 0.2) = 0.2*S + 0.8*relu(S)
and softmax over n is invariant to adding any g(r), so the 0.2*f1[r] term is
dropped.  Each core therefore computes (transposed layout: n on partitions,
r on the free dim):
    g[n, r]  = relu(0.8*f1[r] + 0.8*(f2[n]+b2))            (DVE, bf16)
    x[n, r]  = g + biasT[n, r]                             (DVE, bf16)
    e[n, r]  = exp(x + 0.2*(f2[n]+b2))                     (one ACT pass)
    valsT    = [sf | 1s].T @ e   -> [O+1, R] rows 0..O-1 = unnormalized vals,
                                    row O = softmax denominators
    out      = elu(valsT[:O] / valsT[O])

Sharding: rows r are sharded across 8 cores (1024 each).  The host supplies
per-core transposed shards (seqT, biasT) so all device DMA is contiguous;
biasT is cast to bf16 in-flight by the SWDGE DMA engines.  seq (bf16) is
replicated; each core computes the full seq_fts in node-major layout
directly (seqTf chunk stationary on the PE, [W1 | W1@a2] moving), which
also yields f2 per node as a free 65th column.
"""

import sys

sys.path.insert(0, "/opt/trn_rl_repo")

import numpy as np
import jax.numpy as jnp

import concourse.bacc as bacc
import concourse.bass as bass
import concourse.mybir as mybir
import concourse.tile as tile
from concourse.bass_utils import run_bass_kernel_spmd

F32 = mybir.dt.float32
F32R = mybir.dt.float32r
BF16 = mybir.dt.bfloat16
ADD = mybir.AluOpType.add
MAX = mybir.AluOpType.max
MIN = mybir.AluOpType.min
MULT = mybir.AluOpType.mult
EXP = mybir.ActivationFunctionType.Exp
COPY = mybir.ActivationFunctionType.Copy

M = 8          # cores
N = 8192       # nodes (columns of the attention matrix)
R = N // M     # rows per core (1024)
F_IN = 256
O = 64
P = 128        # partitions
H = 512        # matmul free-dim half (PSUM bank limit)

_CACHED = {}


def build_nc(n=N, r=R, bt_bufs=24):
    nch = n // P           # chunks (128 nodes each)
    nblk = n // r          # seq_fts column-blocks (emitted one ahead)
    cpb = nch // nblk      # chunks per block
    hs = [slice(i * H, min((i + 1) * H, r)) for i in range((r + H - 1) // H)]

    nc = bacc.Bacc(
        "TRN2",
        target_bir_lowering=False,
        debug=False,
        enable_asserts=True,
        num_devices=M,
    )

    seqT_d = nc.dram_tensor("seqT", [F_IN, r], BF16, kind="ExternalInput")
    seqTf_d = nc.dram_tensor("seqTf", [F_IN, n], BF16, kind="ExternalInput")
    biasT_d = nc.dram_tensor("biasT", [n, r], F32R, kind="ExternalInput")
    W1_d = nc.dram_tensor("W1", [F_IN, O], BF16, kind="ExternalInput")
    W1T_d = nc.dram_tensor("W1T", [O, F_IN], BF16, kind="ExternalInput")
    a1_d = nc.dram_tensor("a1", [O, 1], BF16, kind="ExternalInput")
    a2_d = nc.dram_tensor("a2", [O, 1], BF16, kind="ExternalInput")
    b1_d = nc.dram_tensor("b1", [1, 1], F32, kind="ExternalInput")
    b2p_d = nc.dram_tensor("b2p", [P, 1], F32, kind="ExternalInput")
    onespb_d = nc.dram_tensor("onespb", [1, P], BF16, kind="ExternalInput")
    oneso_d = nc.dram_tensor("oneso", [1, O], F32, kind="ExternalInput")
    out_d = nc.dram_tensor("out", [O, r], F32, kind="ExternalOutput")

    with tile.TileContext(nc) as tc:
        with (
            tc.tile_pool(name="const", bufs=1) as cp,
            tc.tile_pool(name="bt", bufs=bt_bufs) as btp,
            tc.tile_pool(name="g", bufs=3) as gp,
            tc.tile_pool(name="x", bufs=3) as xsp,
            tc.tile_pool(name="e", bufs=3) as ep,
            tc.tile_pool(name="small", bufs=4) as smp,
            tc.tile_pool(name="blk", bufs=4) as bkp,
            tc.tile_pool(name="vp", bufs=1, space="PSUM") as vp,
            tc.tile_pool(name="sp", bufs=2, space="PSUM") as sp,
        ):
            # ---- constants / inputs ----
            onespb = cp.tile([1, P], BF16)
            nc.scalar.dma_start(onespb[:], onespb_d.ap())
            oneso = cp.tile([1, O], F32)
            nc.scalar.dma_start(oneso[:], oneso_d.ap())
            w1a = cp.tile([P, O], BF16)
            nc.scalar.dma_start(w1a[:], W1_d.ap()[0:P, :])
            w1b = cp.tile([P, O], BF16)
            nc.scalar.dma_start(w1b[:], W1_d.ap()[P:F_IN, :])
            w1t = cp.tile([O, F_IN], BF16)
            nc.scalar.dma_start(w1t[:], W1T_d.ap())
            a1b = cp.tile([O, 1], BF16)
            nc.scalar.dma_start(a1b[:], a1_d.ap())
            a2b = cp.tile([O, 1], BF16)
            nc.scalar.dma_start(a2b[:], a2_d.ap())
            b1s = cp.tile([1, 1], F32)
            nc.scalar.dma_start(b1s[:], b1_d.ap())
            b2p = cp.tile([P, 1], F32)
            nc.scalar.dma_start(b2p[:], b2p_d.ap())
            seqta = cp.tile([P, r], BF16)
            nc.scalar.dma_start(seqta[:], seqT_d.ap()[0:P, :])
            seqtb = cp.tile([P, r], BF16)
            nc.scalar.dma_start(seqtb[:], seqT_d.ap()[P:F_IN, :])
            # full transposed seq (bf16), two big DMAs
            sqa = cp.tile([P, n], BF16)
            nc.scalar.dma_start(sqa[:], seqTf_d.ap()[0:P, :])
            sqb = cp.tile([P, n], BF16)
            nc.scalar.dma_start(sqb[:], seqTf_d.ap()[P:F_IN, :])

            # ---- w2 = W1 @ a2, separate [P, 1] weight columns ----
            w2a_ps = sp.tile([P, 1], F32, tag="scratch")
            nc.tensor.matmul(w2a_ps[:], w1t[:, 0:P], a2b[:], start=True, stop=True)
            w2b_ps = sp.tile([P, 1], F32, tag="scratch")
            nc.tensor.matmul(w2b_ps[:], w1t[:, P:F_IN], a2b[:], start=True, stop=True)
            w2a = cp.tile([P, 1], BF16)
            w2b = cp.tile([P, 1], BF16)
            with nc.allow_low_precision(reason="f2 weight column, feeds exp"):
                nc.scalar.activation(w2a[:], w2a_ps[:], COPY)
                nc.scalar.activation(w2b[:], w2b_ps[:], COPY)

            # ---- own-shard seq_fts, f1, F1B = 0.8*(f1+b1) broadcast (bf16) ----
            sft_ps = sp.tile([O, r], F32, tag="scratch")
            for sl in hs:
                nc.tensor.matmul(sft_ps[:, sl], w1a[:], seqta[:, sl], start=True, stop=False)
            for sl in hs:
                nc.tensor.matmul(sft_ps[:, sl], w1b[:], seqtb[:, sl], start=False, stop=True)
            sft_b = cp.tile([O, r], BF16)
            nc.scalar.activation(sft_b[:], sft_ps[:], COPY)

            f1_ps = sp.tile([1, r], F32, tag="scratch")
            for sl in hs:
                nc.tensor.matmul(f1_ps[:, sl], a1b[:], sft_b[:, sl], start=True, stop=True)
            f1row = cp.tile([1, r], BF16)
            with nc.allow_low_precision(reason="logit scale, feeds exp"):
                nc.vector.tensor_scalar(f1row[:], f1_ps[:], b1s[:], 0.8, op0=ADD, op1=MULT)

            f1b_ps = sp.tile([P, r], F32, tag="scratch")
            for sl in hs:
                nc.tensor.matmul(f1b_ps[:, sl], onespb[:], f1row[:, sl], start=True, stop=True)
            f1b = cp.tile([P, r], BF16)
            with nc.allow_low_precision(reason="logit scale, feeds exp"):
                nc.scalar.activation(f1b[:], f1b_ps[:], COPY)

            # ---- seq_fts full (node-major, augmented with ones col) ----
            sfaug = cp.tile([P, nch * (O + 1)], BF16)
            sfaug_r3 = sfaug[:].rearrange("p (c o) -> p c o", o=O + 1)
            nc.vector.memset(sfaug_r3[:, :, O : O + 1], 1.0)

            f2ab = {}

            def emit_block(kb):
                # seq_fts rows [kb*cpb*P, (kb+1)*cpb*P) directly in node-major
                # layout: seqTf chunk stationary, W1 half moving.  f2raw for
                # the block's nodes lands in the dense tail [cpb*O, cpb*O+cpb)
                # via extra N=1 matmuls against w2, so every PSUM read below
                # is a dense contiguous range.
                sfb_ps = sp.tile([P, cpb * (O + 1)], F32, tag="scratch")
                fbase = cpb * O
                for j in range(cpb):
                    cc = kb * cpb + j
                    csl = slice(cc * P, (cc + 1) * P)
                    osl = slice(j * O, (j + 1) * O)
                    fsl = slice(fbase + j, fbase + j + 1)
                    nc.tensor.matmul(sfb_ps[:, osl], sqa[:, csl], w1a[:], start=True, stop=False)
                    nc.tensor.matmul(sfb_ps[:, fsl], sqa[:, csl], w2a[:], start=True, stop=False)
                    nc.tensor.matmul(sfb_ps[:, osl], sqb[:, csl], w1b[:], start=False, stop=True)
                    nc.tensor.matmul(sfb_ps[:, fsl], sqb[:, csl], w2b[:], start=False, stop=True)
                with nc.allow_low_precision(reason="seq_fts reused as bf16 weights"):
                    nc.scalar.activation(
                        sfaug_r3[:, kb * cpb : (kb + 1) * cpb, 0:O],
                        sfb_ps[:, 0 : cpb * O].rearrange("p (j o) -> p j o", o=O),
                        COPY,
                    )
                # f2a = 0.8*(f2+b2), f2b = 0.2*(f2+b2) for the whole block
                f2a_blk = bkp.tile([P, cpb], F32, tag="f2a")
                nc.vector.tensor_scalar(
                    f2a_blk[:], sfb_ps[:, fbase : fbase + cpb], b2p[:], 0.8, op0=ADD, op1=MULT
                )
                f2b_blk = bkp.tile([P, cpb], F32, tag="f2b")
                nc.vector.tensor_scalar(f2b_blk[:], f2a_blk[:], 0.25, None, op0=MULT)
                f2ab[kb] = (f2a_blk, f2b_blk)

            emit_block(0)

            # ---- main loop ----
            vals = vp.tile([O + 1, r], F32)
            for c in range(nch):
                if c % cpb == 0 and c // cpb + 1 < nblk:
                    emit_block(c // cpb + 1)
                kb, j = divmod(c, cpb)
                f2a_blk, f2b_blk = f2ab[kb]

                bt = btp.tile([P, r], F32R)
                nc.sync.dma_start(bt[:], biasT_d.ap()[c * P : (c + 1) * P, :])

                # g = relu(0.8*f1 + 0.8*(f2+b2))  (bf16)
                g = gp.tile([P, r], BF16)
                with nc.allow_low_precision(reason="logit scale, feeds exp"):
                    nc.vector.tensor_scalar(
                        g[:], f1b[:], f2a_blk[:, j : j + 1], 0.0, op0=ADD, op1=MAX
                    )
                # x = (g + 0.2*(f2+b2)) + bias  (bf16; folds the exp bias in)
                x = xsp.tile([P, r], BF16)
                with nc.allow_low_precision(reason="logit scale, feeds exp"):
                    nc.vector.scalar_tensor_tensor(
                        x[:], g[:], f2b_blk[:, j : j + 1], bt[:].bitcast(F32),
                        op0=ADD, op1=ADD,
                    )

                e = ep.tile([P, r], BF16)
                nc.scalar.activation(e[:], x[:], EXP)

                csl65 = slice(c * (O + 1), (c + 1) * (O + 1))
                for sl in hs:
                    nc.tensor.matmul(
                        vals[:, sl],
                        sfaug[:, csl65],
                        e[:, sl],
                        start=(c == 0),
                        stop=(c == nch - 1),
                    )

            # ---- epilogue: divide by row sums, elu, out ----
            den = cp.tile([1, r], F32)
            nc.scalar.activation(den[:], vals[O : O + 1, :], COPY)
            recip = cp.tile([1, r], F32)
            nc.vector.reciprocal_approx_fast(recip[:], den[:])
            rb_ps = sp.tile([O, r], F32, tag="scratch")
            for sl in hs:
                nc.tensor.matmul(
                    rb_ps[:, sl], oneso[:], recip[:, sl],
                    start=True, stop=True,
                )
            rb = cp.tile([O, r], F32)
            nc.scalar.activation(rb[:], rb_ps[:], COPY)
            vn = cp.tile([O, r], F32)
            nc.vector.tensor_mul(vn[:], vals[0:O, :], rb[:])
            # elu(x) = (relu(x) - 1) + exp(min(x, 0))
            p2 = cp.tile([O, r], F32)
            nc.vector.tensor_scalar(p2[:], vn[:], 0.0, -1.0, op0=MAX, op1=ADD)
            mn = cp.tile([O, r], F32)
            nc.vector.tensor_scalar(mn[:], vn[:], 0.0, None, op0=MIN)
            em = cp.tile([O, r], F32)
            nc.scalar.activation(em[:], mn[:], EXP)
            outT = cp.tile([O, r], F32)
            nc.vector.tensor_add(outT[:], p2[:], em[:])
            nc.scalar.dma_start(out_d.ap(), outT[:])

    nc.compile()
    return nc


def get_nc():
    if "nc" not in _CACHED:
        _CACHED["nc"] = build_nc()
    return _CACHED["nc"]


def _bf16(a):
    return np.asarray(jnp.asarray(np.asarray(a, np.float32), jnp.bfloat16))


def make_in_maps(seq, bias_mat, W1, a1, b1, a2, b2, n=N, r=R):
    m = n // r
    seq2 = np.asarray(seq, dtype=np.float32).reshape(n, F_IN)
    bias2 = np.asarray(bias_mat, dtype=np.float32).reshape(n, n)
    seqTf = _bf16(np.ascontiguousarray(seq2.T))
    W1f = np.asarray(W1, np.float32).reshape(F_IN, O)
    b2f = np.float32(np.asarray(b2).reshape(()))
    common = {
        "seqTf": seqTf,
        "W1": _bf16(W1f),
        "W1T": _bf16(np.ascontiguousarray(W1f.T)),
        "a1": _bf16(np.asarray(a1, np.float32).reshape(O, 1)),
        "a2": _bf16(np.asarray(a2, np.float32).reshape(O, 1)),
        "b1": np.asarray(b1, np.float32).reshape(1, 1),
        "b2p": np.full((P, 1), b2f, np.float32),
        "onespb": _bf16(np.ones((1, P))),
        "oneso": np.ones((1, O), np.float32),
    }
    in_maps = []
    for i in range(m):
        rows = slice(i * r, (i + 1) * r)
        in_maps.append(
            dict(
                common,
                seqT=_bf16(np.ascontiguousarray(seq2[rows, :].T)),
                biasT=np.ascontiguousarray(bias2[rows, :].T),
            )
        )
    return in_maps


def kernel(seq, bias_mat, W1, a1, b1, a2, b2):
    nc = get_nc()
    in_maps = make_in_maps(seq, bias_mat, W1, a1, b1, a2, b2)
    res = run_bass_kernel_spmd(nc, in_maps, core_ids=list(range(M)))
    outs = [res.results[i]["out"] for i in range(M)]
    full = np.concatenate([o.T for o in outs], axis=0)  # [N, O]
    return full.reshape(1, N, O).astype(np.float32)


if __name__ == "__main__":
    rng = np.random.default_rng(0)
    seq = rng.standard_normal((1, N, F_IN), dtype=np.float32)
    bias = np.zeros((1, N, N), np.float32)
    W1 = (rng.standard_normal((F_IN, O)) * 0.05).astype(np.float32)
    a1 = (rng.standard_normal((O, 1)) * 0.05).astype(np.float32)
    a2 = (rng.standard_normal((O, 1)) * 0.05).astype(np.float32)
    b1 = np.zeros((1,), np.float32)
    b2 = np.zeros((1,), np.float32)
    out = kernel(seq=seq, bias_mat=bias, W1=W1, a1=a1, b1=b1, a2=a2, b2=b2)
    print(out.shape, out.dtype)


# revision 17
# speedup vs baseline: 1.1641x; 1.1641x over previous
"""GAT attention head (single head) distributed across 8 TRN2 NeuronCores.

Math (reference):
    sf   = seq @ W1                        # [N, O]
    f1   = sf @ a1 + b1                    # [N, 1]
    f2   = sf @ a2 + b2                    # [N, 1]
    lg   = f1 + f2.T                       # [N, N]
    co   = softmax(leaky_relu(lg, 0.2) + bias, axis=-1)
    out  = elu(co @ sf)                    # [N, O]

Key algebraic transform: with S = f1[r] + f2[n],
    leaky_relu(S, 0.2) = 0.2*S + 0.8*relu(S)
and softmax over n is invariant to adding any g(r), so the 0.2*f1[r] term is
dropped.  Each core therefore computes (transposed layout: n on partitions,
r on the free dim):
    g[n, r]  = relu(0.8*f1[r] + 0.8*(f2[n]+b2))            (DVE, bf16)
    x[n, r]  = g + biasT[n, r]                             (DVE, bf16)
    e[n, r]  = exp(x + 0.2*(f2[n]+b2))                     (one ACT pass)
    valsT    = [sf | 1s].T @ e   -> [O+1, R] rows 0..O-1 = unnormalized vals,
                                    row O = softmax denominators
    out      = elu(valsT[:O] / valsT[O])

Sharding: rows r are sharded across 8 cores (1024 each).  The host supplies
per-core transposed shards (seqT, biasT) so all device DMA is contiguous;
biasT is cast to bf16 in-flight by the SWDGE DMA engines.  seq (bf16) is
replicated; each core computes the full seq_fts in node-major layout
directly (seqTf chunk stationary on the PE, [W1 | W1@a2] moving), which
also yields f2 per node as a free 65th column.
"""

import sys

sys.path.insert(0, "/opt/trn_rl_repo")

import numpy as np
import jax.numpy as jnp

import concourse.bacc as bacc
import concourse.bass as bass
import concourse.mybir as mybir
import concourse.tile as tile
from concourse.bass_utils import run_bass_kernel_spmd

F32 = mybir.dt.float32
F32R = mybir.dt.float32r
BF16 = mybir.dt.bfloat16
ADD = mybir.AluOpType.add
MAX = mybir.AluOpType.max
MIN = mybir.AluOpType.min
MULT = mybir.AluOpType.mult
EXP = mybir.ActivationFunctionType.Exp
COPY = mybir.ActivationFunctionType.Copy

M = 8          # cores
N = 8192       # nodes (columns of the attention matrix)
R = N // M     # rows per core (1024)
F_IN = 256
O = 64
P = 128        # partitions
H = 512        # matmul free-dim half (PSUM bank limit)

_CACHED = {}


def build_nc(n=N, r=R, bt_bufs=24):
    nch = n // P           # chunks (128 nodes each)
    nblk = n // r          # seq_fts column-blocks (emitted one ahead)
    cpb = nch // nblk      # chunks per block
    hs = [slice(i * H, min((i + 1) * H, r)) for i in range((r + H - 1) // H)]

    nc = bacc.Bacc(
        "TRN2",
        target_bir_lowering=False,
        debug=False,
        enable_asserts=True,
        num_devices=M,
    )

    seqT_d = nc.dram_tensor("seqT", [F_IN, r], BF16, kind="ExternalInput")
    seqTf_d = nc.dram_tensor("seqTf", [F_IN, n], BF16, kind="ExternalInput")
    biasT_d = nc.dram_tensor("biasT", [n, r], F32R, kind="ExternalInput")
    W1_d = nc.dram_tensor("W1", [F_IN, O], BF16, kind="ExternalInput")
    W1T_d = nc.dram_tensor("W1T", [O, F_IN], BF16, kind="ExternalInput")
    a1_d = nc.dram_tensor("a1", [O, 1], BF16, kind="ExternalInput")
    a2_d = nc.dram_tensor("a2", [O, 1], BF16, kind="ExternalInput")
    b1_d = nc.dram_tensor("b1", [1, 1], F32, kind="ExternalInput")
    b2p_d = nc.dram_tensor("b2p", [P, 1], F32, kind="ExternalInput")
    onespb_d = nc.dram_tensor("onespb", [1, P], BF16, kind="ExternalInput")
    oneso_d = nc.dram_tensor("oneso", [1, O], F32, kind="ExternalInput")
    out_d = nc.dram_tensor("out", [O, r], F32, kind="ExternalOutput")

    with tile.TileContext(nc) as tc:
        with (
            tc.tile_pool(name="const", bufs=1) as cp,
            tc.tile_pool(name="bt", bufs=bt_bufs) as btp,
            tc.tile_pool(name="g", bufs=3) as gp,
            tc.tile_pool(name="x", bufs=3) as xsp,
            tc.tile_pool(name="e", bufs=3) as ep,
            tc.tile_pool(name="small", bufs=4) as smp,
            tc.tile_pool(name="blk", bufs=4) as bkp,
            tc.tile_pool(name="vp", bufs=1, space="PSUM") as vp,
            tc.tile_pool(name="sp", bufs=2, space="PSUM") as sp,
        ):
            # ---- constants / inputs ----
            onespb = cp.tile([1, P], BF16)
            nc.scalar.dma_start(onespb[:], onespb_d.ap())
            oneso = cp.tile([1, O], F32)
            nc.scalar.dma_start(oneso[:], oneso_d.ap())
            w1a = cp.tile([P, O], BF16)
            nc.scalar.dma_start(w1a[:], W1_d.ap()[0:P, :])
            w1b = cp.tile([P, O], BF16)
            nc.scalar.dma_start(w1b[:], W1_d.ap()[P:F_IN, :])
            w1t = cp.tile([O, F_IN], BF16)
            nc.scalar.dma_start(w1t[:], W1T_d.ap())
            a1b = cp.tile([O, 1], BF16)
            nc.scalar.dma_start(a1b[:], a1_d.ap())
            a2b = cp.tile([O, 1], BF16)
            nc.scalar.dma_start(a2b[:], a2_d.ap())
            b1s = cp.tile([1, 1], F32)
            nc.scalar.dma_start(b1s[:], b1_d.ap())
            b2p = cp.tile([P, 1], F32)
            nc.scalar.dma_start(b2p[:], b2p_d.ap())
            seqta = cp.tile([P, r], BF16)
            nc.scalar.dma_start(seqta[:], seqT_d.ap()[0:P, :])
            seqtb = cp.tile([P, r], BF16)
            nc.scalar.dma_start(seqtb[:], seqT_d.ap()[P:F_IN, :])
            # full transposed seq (bf16), two big DMAs
            sqa = cp.tile([P, n], BF16)
            nc.scalar.dma_start(sqa[:], seqTf_d.ap()[0:P, :])
            sqb = cp.tile([P, n], BF16)
            nc.scalar.dma_start(sqb[:], seqTf_d.ap()[P:F_IN, :])

            # ---- w2 = W1 @ a2, separate [P, 1] weight columns ----
            w2a_ps = sp.tile([P, 1], F32, tag="scratch")
            nc.tensor.matmul(w2a_ps[:], w1t[:, 0:P], a2b[:], start=True, stop=True)
            w2b_ps = sp.tile([P, 1], F32, tag="scratch")
            nc.tensor.matmul(w2b_ps[:], w1t[:, P:F_IN], a2b[:], start=True, stop=True)
            w2a = cp.tile([P, 1], BF16)
            w2b = cp.tile([P, 1], BF16)
            with nc.allow_low_precision(reason="f2 weight column, feeds exp"):
                nc.scalar.activation(w2a[:], w2a_ps[:], COPY)
                nc.scalar.activation(w2b[:], w2b_ps[:], COPY)

            # ---- own-shard seq_fts, f1, F1B = 0.8*(f1+b1) broadcast (bf16) ----
            sft_ps = sp.tile([O, r], F32, tag="scratch")
            for sl in hs:
                nc.tensor.matmul(sft_ps[:, sl], w1a[:], seqta[:, sl], start=True, stop=False)
            for sl in hs:
                nc.tensor.matmul(sft_ps[:, sl], w1b[:], seqtb[:, sl], start=False, stop=True)
            sft_b = cp.tile([O, r], BF16)
            nc.scalar.activation(sft_b[:], sft_ps[:], COPY)

            f1_ps = sp.tile([1, r], F32, tag="scratch")
            for sl in hs:
                nc.tensor.matmul(f1_ps[:, sl], a1b[:], sft_b[:, sl], start=True, stop=True)
            f1row = cp.tile([1, r], BF16)
            with nc.allow_low_precision(reason="logit scale, feeds exp"):
                nc.vector.tensor_scalar(f1row[:], f1_ps[:], b1s[:], 0.8, op0=ADD, op1=MULT)

            f1b_ps = sp.tile([P, r], F32, tag="scratch")
            for sl in hs:
                nc.tensor.matmul(f1b_ps[:, sl], onespb[:], f1row[:, sl], start=True, stop=True)
            f1b = cp.tile([P, r], BF16)
            with nc.allow_low_precision(reason="logit scale, feeds exp"):
                nc.scalar.activation(f1b[:], f1b_ps[:], COPY)

            # ---- seq_fts full (node-major, augmented with ones col) ----
            sfaug = cp.tile([P, nch * (O + 1)], BF16)
            sfaug_r3 = sfaug[:].rearrange("p (c o) -> p c o", o=O + 1)
            nc.vector.memset(sfaug_r3[:, :, O : O + 1], 1.0)

            f2ab = {}

            def emit_block(kb):
                # seq_fts rows [kb*cpb*P, (kb+1)*cpb*P) directly in node-major
                # layout: seqTf chunk stationary, W1 half moving.  f2raw for
                # the block's nodes lands in the dense tail [cpb*O, cpb*O+cpb)
                # via extra N=1 matmuls against w2, so every PSUM read below
                # is a dense contiguous range.
                # f2 tail starts at a PSUM-bank boundary (512 f32) so its
                # accumulation group's zero region is disjoint from the
                # seq_fts groups'.
                fbase = ((cpb * O + H - 1) // H) * H
                sfb_ps = sp.tile([P, fbase + cpb], F32, tag="scratch")
                for j in range(cpb):
                    cc = kb * cpb + j
                    csl = slice(cc * P, (cc + 1) * P)
                    osl = slice(j * O, (j + 1) * O)
                    nc.tensor.matmul(sfb_ps[:, osl], sqa[:, csl], w1a[:], start=True, stop=False)
                    nc.tensor.matmul(sfb_ps[:, osl], sqb[:, csl], w1b[:], start=False, stop=True)
                for j in range(cpb):
                    cc = kb * cpb + j
                    csl = slice(cc * P, (cc + 1) * P)
                    fsl = slice(fbase + j, fbase + j + 1)
                    nc.tensor.matmul(
                        sfb_ps[:, fsl], sqa[:, csl], w2a[:],
                        start=(j == 0), stop=False, skip_group_check=True,
                    )
                    nc.tensor.matmul(
                        sfb_ps[:, fsl], sqb[:, csl], w2b[:],
                        start=False, stop=(j == cpb - 1), skip_group_check=True,
                    )
                with nc.allow_low_precision(reason="seq_fts reused as bf16 weights"):
                    nc.scalar.activation(
                        sfaug_r3[:, kb * cpb : (kb + 1) * cpb, 0:O],
                        sfb_ps[:, 0 : cpb * O].rearrange("p (j o) -> p j o", o=O),
                        COPY,
                    )
                # f2a = 0.8*(f2+b2), f2b = 0.2*(f2+b2) for the whole block
                f2a_blk = bkp.tile([P, cpb], F32, tag="f2a")
                nc.vector.tensor_scalar(
                    f2a_blk[:], sfb_ps[:, fbase : fbase + cpb], b2p[:], 0.8, op0=ADD, op1=MULT
                )
                f2b_blk = bkp.tile([P, cpb], F32, tag="f2b")
                nc.vector.tensor_scalar(f2b_blk[:], f2a_blk[:], 0.25, None, op0=MULT)
                f2ab[kb] = (f2a_blk, f2b_blk)

            emit_block(0)

            # ---- main loop ----
            vals = vp.tile([O + 1, r], F32)
            for c in range(nch):
                if c % cpb == 0 and c // cpb + 1 < nblk:
                    emit_block(c // cpb + 1)
                kb, j = divmod(c, cpb)
                f2a_blk, f2b_blk = f2ab[kb]

                bt = btp.tile([P, r], F32R)
                nc.sync.dma_start(bt[:], biasT_d.ap()[c * P : (c + 1) * P, :])

                # g = relu(0.8*f1 + 0.8*(f2+b2))  (bf16)
                g = gp.tile([P, r], BF16)
                with nc.allow_low_precision(reason="logit scale, feeds exp"):
                    nc.vector.tensor_scalar(
                        g[:], f1b[:], f2a_blk[:, j : j + 1], 0.0, op0=ADD, op1=MAX
                    )
                # x = (g + 0.2*(f2+b2)) + bias  (bf16; folds the exp bias in)
                x = xsp.tile([P, r], BF16)
                with nc.allow_low_precision(reason="logit scale, feeds exp"):
                    nc.vector.scalar_tensor_tensor(
                        x[:], g[:], f2b_blk[:, j : j + 1], bt[:].bitcast(F32),
                        op0=ADD, op1=ADD,
                    )

                e = ep.tile([P, r], BF16)
                nc.scalar.activation(e[:], x[:], EXP)

                csl65 = slice(c * (O + 1), (c + 1) * (O + 1))
                for sl in hs:
                    nc.tensor.matmul(
                        vals[:, sl],
                        sfaug[:, csl65],
                        e[:, sl],
                        start=(c == 0),
                        stop=(c == nch - 1),
                    )

            # ---- epilogue: divide by row sums, elu, out ----
            den = cp.tile([1, r], F32)
            nc.scalar.activation(den[:], vals[O : O + 1, :], COPY)
            recip = cp.tile([1, r], F32)
            nc.vector.reciprocal_approx_fast(recip[:], den[:])
            rb_ps = sp.tile([O, r], F32, tag="scratch")
            for sl in hs:
                nc.tensor.matmul(
                    rb_ps[:, sl], oneso[:], recip[:, sl],
                    start=True, stop=True,
                )
            rb = cp.tile([O, r], F32)
            nc.scalar.activation(rb[:], rb_ps[:], COPY)
            vn = cp.tile([O, r], F32)
            nc.vector.tensor_mul(vn[:], vals[0:O, :], rb[:])
            # elu(x) = (relu(x) - 1) + exp(min(x, 0))
            p2 = cp.tile([O, r], F32)
            nc.vector.tensor_scalar(p2[:], vn[:], 0.0, -1.0, op0=MAX, op1=ADD)
            mn = cp.tile([O, r], F32)
            nc.vector.tensor_scalar(mn[:], vn[:], 0.0, None, op0=MIN)
            em = cp.tile([O, r], F32)
            nc.scalar.activation(em[:], mn[:], EXP)
            outT = cp.tile([O, r], F32)
            nc.vector.tensor_add(outT[:], p2[:], em[:])
            nc.scalar.dma_start(out_d.ap(), outT[:])

    nc.compile()
    return nc


def get_nc():
    if "nc" not in _CACHED:
        _CACHED["nc"] = build_nc()
    return _CACHED["nc"]


def _bf16(a):
    return np.asarray(jnp.asarray(np.asarray(a, np.float32), jnp.bfloat16))


def make_in_maps(seq, bias_mat, W1, a1, b1, a2, b2, n=N, r=R):
    m = n // r
    seq2 = np.asarray(seq, dtype=np.float32).reshape(n, F_IN)
    bias2 = np.asarray(bias_mat, dtype=np.float32).reshape(n, n)
    seqTf = _bf16(np.ascontiguousarray(seq2.T))
    W1f = np.asarray(W1, np.float32).reshape(F_IN, O)
    b2f = np.float32(np.asarray(b2).reshape(()))
    common = {
        "seqTf": seqTf,
        "W1": _bf16(W1f),
        "W1T": _bf16(np.ascontiguousarray(W1f.T)),
        "a1": _bf16(np.asarray(a1, np.float32).reshape(O, 1)),
        "a2": _bf16(np.asarray(a2, np.float32).reshape(O, 1)),
        "b1": np.asarray(b1, np.float32).reshape(1, 1),
        "b2p": np.full((P, 1), b2f, np.float32),
        "onespb": _bf16(np.ones((1, P))),
        "oneso": np.ones((1, O), np.float32),
    }
    in_maps = []
    for i in range(m):
        rows = slice(i * r, (i + 1) * r)
        in_maps.append(
            dict(
                common,
                seqT=_bf16(np.ascontiguousarray(seq2[rows, :].T)),
                biasT=np.ascontiguousarray(bias2[rows, :].T),
            )
        )
    return in_maps


def kernel(seq, bias_mat, W1, a1, b1, a2, b2):
    nc = get_nc()
    in_maps = make_in_maps(seq, bias_mat, W1, a1, b1, a2, b2)
    res = run_bass_kernel_spmd(nc, in_maps, core_ids=list(range(M)))
    outs = [res.results[i]["out"] for i in range(M)]
    full = np.concatenate([o.T for o in outs], axis=0)  # [N, O]
    return full.reshape(1, N, O).astype(np.float32)


if __name__ == "__main__":
    rng = np.random.default_rng(0)
    seq = rng.standard_normal((1, N, F_IN), dtype=np.float32)
    bias = np.zeros((1, N, N), np.float32)
    W1 = (rng.standard_normal((F_IN, O)) * 0.05).astype(np.float32)
    a1 = (rng.standard_normal((O, 1)) * 0.05).astype(np.float32)
    a2 = (rng.standard_normal((O, 1)) * 0.05).astype(np.float32)
    b1 = np.zeros((1,), np.float32)
    b2 = np.zeros((1,), np.float32)
    out = kernel(seq=seq, bias_mat=bias, W1=W1, a1=a1, b1=b1, a2=a2, b2=b2)
    print(out.shape, out.dtype)
